# revision 1
# baseline (speedup 1.0000x reference)
"""LRN (Local Response Normalization, TF-style cross-W+C window) Trainium2 kernel.

Reference computation (on [B,H,W,C] = [32,224,224,64] f32):
    s[b,h,w]   = sum_c x[b,h,w,c]^2
    win[b,h,w] = sum_{d=-5..5} s[b,h,w+d]        (zero-padded SAME over W)
    out        = x / sqrt(1 + 1.0*win)           (bias=1, alpha=1, beta=0.5)

Sharding: pure data-parallel over batch. 8 cores x 4 batches each.
Per-core layout: rows = (b,h) pairs -> 896 rows = 7 tiles of 128 partitions,
free axis = (w, c) = 224*64 = 14336 f32 (57 KiB/partition, contiguous in HBM).

Per tile:
  DMA in  [128, 224, 64]
  ACT Square (4 chunks of 56 w)  ->  x2 buffer
  DVE reduce_sum axis=X on [128, 56, 64] -> s_pad[:, 5 + chunk]   (grouped C-sum)
  DVE log-shift adds: w2 = s+s>>1, w4 = w2+w2>>2, w8 = w4+w4>>4,
                      win = w8 + w2>>8 + s_pad>>10          (11-wide window in 5 ops)
  ACT Sqrt(alpha*win + bias) -> denom ; DVE reciprocal -> rstd
  DVE tensor_mul x * rstd (stride-0 broadcast over C), in place
  DMA out [128, 224, 64]
"""

import json
import re

import numpy as np

import concourse.bass as bass
import concourse.tile as tile
from concourse import mybir
from concourse.bass_utils import run_bass_kernel_spmd

# Problem constants (hardcoded per harness contract).
B, H, W, C = 32, 224, 224, 64
N_CORES = 8
RADIUS = 5
KWIN = 2 * RADIUS + 1  # 11
BIAS = 1.0
ALPHA = 1.0

P = 128
B_PER_CORE = B // N_CORES          # 4
ROWS = B_PER_CORE * H              # 896
NTILES = ROWS // P                 # 7
N_WCHUNK = 4
WCH = W // N_WCHUNK                # 56
WPAD = W + KWIN - 1                # 234

_F32 = mybir.dt.float32

# The walrus build in this container accepts only ONE sync-wait slot per TPB
# instruction ("Too many sync wait commands" in setupSyncWait otherwise),
# while Tile's scheduler freely attaches 2-3 waits per instruction. Legalize
# the BIR before compilation: drop same-engine program-order self-waits
# (trivially satisfied on an in-order sequencer) and hoist any remaining
# excess waits onto standalone EventSemaphore instructions just before the
# owning instruction on the same engine.
_ENGINE_SEM = re.compile(r"^(Pool|Activation|PE|DVE|SP)_\d+$")


def _legalize_bir_waits(bir: bytes, max_waits: int = 1) -> bytes:
    d = json.loads(bir)
    incers: dict = {}
    for fn in d["functions"]:
        for bb in fn.get("blocks") or []:
            for ins in bb["instructions"]:
                for u in (ins.get("sync_info") or {}).get("on_update") or []:
                    incers.setdefault(u["id"], set()).add(
                        (ins.get("engine"), ins.get("opcode"))
                    )
    n_ev = 0
    for fn in d["functions"]:
        for bb in fn.get("blocks") or []:
            out = []
            for ins in bb["instructions"]:
                si = ins.get("sync_info")
                waits = (si or {}).get("on_wait") or []
                opcode = ins.get("opcode")
                if (
                    si
                    and len(waits) > max_waits
                    and opcode != "EventSemaphore"
                ):
                    eng = ins.get("engine")
                    kept = []
                    for w in waits:
                        nm = w.get("ant_name", "")
                        srcs = incers.get(w.get("id"), set())
                        if (
                            _ENGINE_SEM.match(nm)
                            and nm.startswith(str(eng) + "_")
                            and srcs
                            and all(
                                e == eng and op != "DMACopy" for e, op in srcs
                            )
                        ):
                            # Same-engine program-order wait: every inc comes
                            # from an earlier instruction on this in-order
                            # engine, so it holds by the time this issues.
                            continue
                        kept.append(w)
                    for w in kept[max_waits:]:
                        n_ev += 1
                        out.append(
                            {
                                "debug": ins.get("debug", 0),
                                "engine": eng,
                                "ins": [],
                                "outs": [],
                                "name": f"evw-{n_ev}",
                                "opcode": "EventSemaphore",
                                "sync_info": {"on_update": [], "on_wait": [w]},
                            }
                        )
                    si["on_wait"] = kept[:max_waits]
                out.append(ins)
            bb["instructions"] = out
    return json.dumps(d).encode()


class _WaitLegalBass(bass.Bass):
    def to_json_bytes(self) -> bytes:
        return _legalize_bir_waits(super().to_json_bytes())


def build_nc() -> bass.Bass:
    nc = _WaitLegalBass(trn_type="TRN2")
    x = nc.dram_tensor("x", [ROWS, W, C], _F32, kind="ExternalInput")
    y = nc.dram_tensor("y", [ROWS, W, C], _F32, kind="ExternalOutput")

    with tile.TileContext(nc) as tc:
        with (
            tc.tile_pool(name="xpool", bufs=2) as xpool,
            tc.tile_pool(name="x2pool", bufs=3) as x2pool,
            tc.tile_pool(name="spool", bufs=2) as spool,
            tc.tile_pool(name="wpool", bufs=2) as wpool,
        ):
            for it in range(NTILES):
                r0 = it * P
                x_tile = xpool.tile([P, W, C], _F32)
                nc.sync.dma_start(out=x_tile, in_=x[r0 : r0 + P])

                # s_pad holds the C-sums with a 5-wide zero border on each side.
                s_pad = spool.tile([P, WPAD], _F32)
                nc.gpsimd.memset(s_pad[:, 0:RADIUS], 0.0)
                nc.gpsimd.memset(s_pad[:, W + RADIUS : WPAD], 0.0)

                for jc in range(N_WCHUNK):
                    w0 = jc * WCH
                    x2 = x2pool.tile([P, WCH, C], _F32)
                    xin = x_tile[:, w0 : w0 + WCH, :]
                    # Square on GPSIMD for 3 of 4 chunks (keeps the walrus
                    # 1-wait-per-ACT-instruction limit out of play and the
                    # DVE under the DMA roofline); DVE takes the last chunk.
                    eng = nc.gpsimd if jc < 3 else nc.vector
                    eng.tensor_mul(x2, xin, xin)
                    nc.vector.reduce_sum(
                        out=s_pad[:, RADIUS + w0 : RADIUS + w0 + WCH],
                        in_=x2,
                        axis=mybir.AxisListType.X,
                    )

                # Sliding-window sum of width 11 via log-shift composition.
                # win[w] = sum_{d=0..10} s_pad[w+d],  w in [0, 224).
                w2 = wpool.tile([P, WPAD - 1], _F32)  # w2[j] = s[j] + s[j+1]
                nc.vector.tensor_add(w2, s_pad[:, 0 : WPAD - 1], s_pad[:, 1:WPAD])
                w4 = wpool.tile([P, WPAD - 3], _F32)  # covers d 0..3
                nc.vector.tensor_add(w4, w2[:, 0 : WPAD - 3], w2[:, 2 : WPAD - 1])
                w8 = wpool.tile([P, WPAD - 7], _F32)  # covers d 0..7
                nc.vector.tensor_add(w8, w4[:, 0 : WPAD - 7], w4[:, 4 : WPAD - 3])
                t10 = wpool.tile([P, W], _F32)  # d 0..7 plus d 8..9
                nc.vector.tensor_add(t10, w8[:, 0:W], w2[:, 8 : 8 + W])
                win = wpool.tile([P, W], _F32)  # plus d 10
                nc.vector.tensor_add(win, t10, s_pad[:, 10 : 10 + W])

                # denom = sqrt(alpha*win + bias); rstd = 1/denom.
                denom = wpool.tile([P, W], _F32)
                nc.scalar.activation(
                    out=denom,
                    in_=win,
                    func=mybir.ActivationFunctionType.Sqrt,
                    bias=BIAS,
                    scale=ALPHA,
                )
                rstd = wpool.tile([P, W], _F32)
                nc.vector.reciprocal(out=rstd, in_=denom)

                # out = x * rstd, broadcast over C (stride-0 innermost axis).
                rstd_ap = rstd[:, :]
                rstd_bcast = bass.AP(
                    tensor=rstd_ap.tensor,
                    offset=rstd_ap.offset,
                    ap=[rstd_ap.ap[0], rstd_ap.ap[1], [0, C]],
                )
                nc.vector.tensor_mul(x_tile, x_tile, rstd_bcast)

                nc.sync.dma_start(out=y[r0 : r0 + P], in_=x_tile)

    return nc


_NC_CACHE: list = [None]


def _get_nc() -> bass.Bass:
    if _NC_CACHE[0] is None:
        _NC_CACHE[0] = build_nc()
    return _NC_CACHE[0]


def run(x: np.ndarray, **kwargs):
    """Run the SPMD kernel on 8 cores. Returns (out, BassKernelResults)."""
    x = np.ascontiguousarray(x, dtype=np.float32)
    assert x.shape == (B, H, W, C)
    nc = _get_nc()
    in_maps = [
        {"x": x[i * B_PER_CORE : (i + 1) * B_PER_CORE].reshape(ROWS, W, C)}
        for i in range(N_CORES)
    ]
    res = run_bass_kernel_spmd(nc, in_maps, core_ids=list(range(N_CORES)), **kwargs)
    outs = [r["y"].reshape(B_PER_CORE, H, W, C) for r in res.results]
    out = np.concatenate(outs, axis=0)
    return out, res


def kernel(x: np.ndarray) -> np.ndarray:
    out, _ = run(x)
    return out


def bench(x: np.ndarray, reps: int = 16, warmup: int = 3) -> dict:
    """Measure steady-state device time per kernel execution.

    Mirrors bass2jax.run_bass_via_pjrt's multi-core path but without buffer
    donation, with inputs pre-staged on device, issuing `reps` back-to-back
    executions so dispatch overhead pipelines with device execution.
    """
    import time

    import jax
    from jax.sharding import Mesh, PartitionSpec
    from jax.experimental.shard_map import shard_map

    from concourse import bass2jax
    from concourse import mybir as _mybir

    x = np.ascontiguousarray(x, dtype=np.float32)
    nc = _get_nc()
    bass2jax.install_neuronx_cc_hook()

    partition_name = (
        nc.partition_id_tensor.name if nc.partition_id_tensor is not None else None
    )
    in_names, out_names, out_avals = [], [], []
    for alloc in nc.m.functions[0].allocations:
        if not isinstance(alloc, _mybir.MemoryLocationSet):
            continue
        name = alloc.memorylocations[0].name
        if alloc.kind == "ExternalInput":
            if name != partition_name:
                in_names.append(name)
        elif alloc.kind == "ExternalOutput":
            out_names.append(name)
            out_avals.append(
                jax.core.ShapedArray(
                    tuple(alloc.tensor_shape), _mybir.dt.np(alloc.dtype)
                )
            )
    n_params = len(in_names)
    all_names = in_names + out_names
    if partition_name is not None:
        all_names = all_names + [partition_name]

    def _one_call(operands):
        outs = bass2jax._bass_exec_p.bind(
            *operands,
            out_avals=tuple(out_avals),
            in_names=tuple(all_names),
            out_names=tuple(out_names),
            lowering_input_output_aliases=(),
            sim_require_finite=True,
            sim_require_nnan=True,
            nc=nc,
        )
        return tuple(outs)

    def _make_body(n_calls):
        # Chain data-dependence (call k+1 consumes call k's output) so XLA
        # cannot CSE the repeated executions; the timing slope over n_calls
        # is then pure on-device kernel time.
        def _body(*args):
            operands = list(args)
            if partition_name is not None:
                operands.append(bass2jax.partition_id_tensor())
            outs = _one_call(operands)
            for _ in range(n_calls - 1):
                chained = [outs[0]] + operands[1:]
                outs = _one_call(chained)
            return tuple(outs)

        return _body

    devices = jax.devices()[:N_CORES]
    mesh = Mesh(np.asarray(devices), ("core",))
    nspec = n_params + len(out_names)

    def _make_fn(n_calls):
        return jax.jit(
            shard_map(
                _make_body(n_calls),
                mesh=mesh,
                in_specs=(PartitionSpec("core"),) * nspec,
                out_specs=(PartitionSpec("core"),) * len(out_names),
                check_rep=False,
            ),
            keep_unused=True,
        )

    xg = x.reshape(N_CORES * ROWS, W, C)
    zeros = [np.zeros((N_CORES * ROWS, W, C), np.float32)]
    sharding = jax.sharding.NamedSharding(mesh, PartitionSpec("core"))
    dev_args = [jax.device_put(a, sharding) for a in [xg] + zeros]

    n_chain = 49
    fn1 = _make_fn(1)
    fnN = _make_fn(n_chain)

    for _ in range(warmup):
        out = fn1(*dev_args)
        outN = fnN(*dev_args)
    jax.block_until_ready(out)
    jax.block_until_ready(outN)

    def _time(f, n):
        best = float("inf")
        for _ in range(n):
            t0 = time.perf_counter()
            jax.block_until_ready(f(*dev_args))
            t1 = time.perf_counter()
            best = min(best, t1 - t0)
        return best

    t1x = _time(fn1, 6)
    tNx = _time(fnN, 4)
    # Slope = pure device time per kernel execution, dispatch excluded.
    device_ns = (tNx - t1x) / (n_chain - 1) * 1e9

    result = np.asarray(fn1(*dev_args)[0]).reshape(B, H, W, C)
    return {
        "device_ns": device_ns,
        "t1_ns": t1x * 1e9,
        "tN_ns": tNx * 1e9,
        "n_chain": n_chain,
        "out": result,
    }



# revision 2
# speedup vs baseline: 1.2557x; 1.2557x over previous
"""LRN (TF-style cross-(W-window, C) local response norm) Trainium2 kernel.

Reference computation (on [B,H,W,C] = [32,224,224,64] f32):
    s[b,h,w]   = sum_c x[b,h,w,c]^2
    win[b,h,w] = sum_{d=-5..5} s[b,h,w+d]        (zero-padded SAME over W)
    out        = x / sqrt(1 + 1.0*win)           (bias=1, alpha=1, beta=0.5)

Sharding: pure data-parallel over batch. 8 cores x 4 batches each.
Per-core layout: rows = (b,h) pairs -> 896 rows = 7 tiles of 128 partitions,
free axis = (w, c) = 224*64 f32 (57 KiB/partition, contiguous in HBM).

Per tile (engine split chosen so the kernel is DMA-bound):
  in-DMA   [128,224,64] in 4 W-chunks (sync/SP HWDGE ring)
  ACT      Square per W-chunk -> x2                      (~3.0us/chunk)
  DVE      reduce_sum axis=X  -> s_pad chunk             (~3.8us/chunk)
  DVE      5 log-shift adds: 11-wide window sum          (~1.4us)
  ACT      denom = Sqrt(1*win + 1)                       (~0.4us)
  DVE      rstd = 1/denom                                (~0.3us)
  DVE      x_tile *= rstd (stride-0 bcast over C), per W-chunk, in place
  out-DMA  per W-chunk as soon as its mul lands (scalar/ACT HWDGE ring)

Engine busy per tile: DMA 40.9us (bound) > DVE ~32us > ACT ~13us; GPSIMD
and PE idle. TimelineSim: 308us/core vs 285us pure-DMA floor (103MB at
~360GB/s per core).
"""

import json
import re
import time

import numpy as np

import concourse.bass as bass
import concourse.tile as tile
from concourse import mybir
from concourse.bass_utils import run_bass_kernel_spmd

# Problem constants (hardcoded per harness contract).
B, H, W, C = 32, 224, 224, 64
N_CORES = 8
RADIUS = 5
KWIN = 2 * RADIUS + 1  # 11
BIAS = 1.0
ALPHA = 1.0

P = 128
B_PER_CORE = B // N_CORES          # 4
ROWS = B_PER_CORE * H              # 896
NTILES = ROWS // P                 # 7
N_WCHUNK = 4
WCH = W // N_WCHUNK                # 56
WPAD = W + KWIN - 1                # 234

_F32 = mybir.dt.float32

# The walrus build in this container accepts only ONE sync-wait slot per TPB
# instruction ("Too many sync wait commands" in setupSyncWait otherwise),
# while Tile's scheduler freely attaches 2-3 waits per instruction. Legalize
# the BIR before compilation: drop same-engine program-order self-waits
# (trivially satisfied on an in-order sequencer) and hoist any remaining
# excess waits onto standalone EventSemaphore instructions just before the
# owning instruction on the same engine.
_ENGINE_SEM = re.compile(r"^(Pool|Activation|PE|DVE|SP)_\d+$")


def _legalize_bir_waits(bir: bytes, max_waits: int = 1) -> bytes:
    d = json.loads(bir)
    incers: dict = {}
    for fn in d["functions"]:
        for bb in fn.get("blocks") or []:
            for ins in bb["instructions"]:
                for u in (ins.get("sync_info") or {}).get("on_update") or []:
                    incers.setdefault(u["id"], set()).add(
                        (ins.get("engine"), ins.get("opcode"))
                    )
    n_ev = 0
    for fn in d["functions"]:
        for bb in fn.get("blocks") or []:
            out = []
            for ins in bb["instructions"]:
                si = ins.get("sync_info")
                waits = (si or {}).get("on_wait") or []
                opcode = ins.get("opcode")
                if (
                    si
                    and len(waits) > max_waits
                    and opcode != "EventSemaphore"
                ):
                    eng = ins.get("engine")
                    kept = []
                    for w in waits:
                        nm = w.get("ant_name", "")
                        srcs = incers.get(w.get("id"), set())
                        if (
                            _ENGINE_SEM.match(nm)
                            and nm.startswith(str(eng) + "_")
                            and srcs
                            and all(
                                e == eng and op != "DMACopy" for e, op in srcs
                            )
                        ):
                            # Same-engine program-order wait: every inc comes
                            # from an earlier instruction on this in-order
                            # engine, so it holds by the time this issues.
                            continue
                        kept.append(w)
                    for w in kept[max_waits:]:
                        n_ev += 1
                        out.append(
                            {
                                "debug": ins.get("debug", 0),
                                "engine": eng,
                                "ins": [],
                                "outs": [],
                                "name": f"evw-{n_ev}",
                                "opcode": "EventSemaphore",
                                "sync_info": {"on_update": [], "on_wait": [w]},
                            }
                        )
                    si["on_wait"] = kept[:max_waits]
                out.append(ins)
            bb["instructions"] = out
    return json.dumps(d).encode()


class _WaitLegalBass(bass.Bass):
    def to_json_bytes(self) -> bytes:
        return _legalize_bir_waits(super().to_json_bytes())


def build_nc() -> bass.Bass:
    nc = _WaitLegalBass(trn_type="TRN2")
    x = nc.dram_tensor("x", [ROWS, W, C], _F32, kind="ExternalInput")
    y = nc.dram_tensor("y", [ROWS, W, C], _F32, kind="ExternalOutput")

    with tile.TileContext(nc) as tc:
        with (
            tc.tile_pool(name="xpool", bufs=2) as xpool,
            tc.tile_pool(name="x2pool", bufs=3) as x2pool,
            tc.tile_pool(name="spool", bufs=2) as spool,
            tc.tile_pool(name="wpool", bufs=2) as wpool,
        ):
            for it in range(NTILES):
                r0 = it * P
                x_tile = xpool.tile([P, W, C], _F32)
                # Chunked load: first Square can start after 1/4 of the tile.
                for jc in range(N_WCHUNK):
                    w0 = jc * WCH
                    nc.sync.dma_start(
                        out=x_tile[:, w0 : w0 + WCH, :],
                        in_=x[r0 : r0 + P, w0 : w0 + WCH, :],
                    )

                # s_pad holds the C-sums with a 5-wide zero border each side.
                s_pad = spool.tile([P, WPAD], _F32)
                nc.vector.memset(s_pad[:, 0:RADIUS], 0.0)
                nc.vector.memset(s_pad[:, W + RADIUS : WPAD], 0.0)

                for jc in range(N_WCHUNK):
                    w0 = jc * WCH
                    x2 = x2pool.tile([P, WCH, C], _F32)
                    nc.scalar.activation(
                        out=x2,
                        in_=x_tile[:, w0 : w0 + WCH, :],
                        func=mybir.ActivationFunctionType.Square,
                    )
                    nc.vector.reduce_sum(
                        out=s_pad[:, RADIUS + w0 : RADIUS + w0 + WCH],
                        in_=x2,
                        axis=mybir.AxisListType.X,
                    )

                # Sliding-window sum of width 11 via log-shift composition.
                # win[w] = sum_{d=0..10} s_pad[w+d],  w in [0, 224).
                w2 = wpool.tile([P, WPAD - 1], _F32)  # w2[j] = s[j] + s[j+1]
                nc.vector.tensor_add(w2, s_pad[:, 0 : WPAD - 1], s_pad[:, 1:WPAD])
                w4 = wpool.tile([P, WPAD - 3], _F32)  # covers d 0..3
                nc.vector.tensor_add(w4, w2[:, 0 : WPAD - 3], w2[:, 2 : WPAD - 1])
                w8 = wpool.tile([P, WPAD - 7], _F32)  # covers d 0..7
                nc.vector.tensor_add(w8, w4[:, 0 : WPAD - 7], w4[:, 4 : WPAD - 3])
                t10 = wpool.tile([P, W], _F32)  # d 0..7 plus d 8..9
                nc.vector.tensor_add(t10, w8[:, 0:W], w2[:, 8 : 8 + W])
                win = wpool.tile([P, W], _F32)  # plus d 10
                nc.vector.tensor_add(win, t10, s_pad[:, 10 : 10 + W])

                # denom = sqrt(alpha*win + bias); rstd = 1/denom.
                denom = wpool.tile([P, W], _F32)
                nc.scalar.activation(
                    out=denom,
                    in_=win,
                    func=mybir.ActivationFunctionType.Sqrt,
                    bias=BIAS,
                    scale=ALPHA,
                )
                rstd = wpool.tile([P, W], _F32)
                nc.vector.reciprocal(out=rstd, in_=denom)

                # out = x * rstd broadcast over C; store each chunk as soon
                # as its multiply lands, on the second HWDGE ring.
                for jc in range(N_WCHUNK):
                    w0 = jc * WCH
                    xc = x_tile[:, w0 : w0 + WCH, :]
                    r_ap = rstd[:, w0 : w0 + WCH]
                    r_bcast = bass.AP(
                        tensor=r_ap.tensor,
                        offset=r_ap.offset,
                        ap=[r_ap.ap[0], r_ap.ap[1], [0, C]],
                    )
                    nc.vector.tensor_mul(xc, xc, r_bcast)
                    nc.scalar.dma_start(
                        out=y[r0 : r0 + P, w0 : w0 + WCH, :], in_=xc
                    )

    return nc


_NC_CACHE: list = [None]


def _get_nc() -> bass.Bass:
    if _NC_CACHE[0] is None:
        _NC_CACHE[0] = build_nc()
    return _NC_CACHE[0]


def run(x: np.ndarray, **kwargs):
    """Run the SPMD kernel on 8 cores. Returns (out, BassKernelResults)."""
    x = np.ascontiguousarray(x, dtype=np.float32)
    assert x.shape == (B, H, W, C)
    nc = _get_nc()
    in_maps = [
        {"x": x[i * B_PER_CORE : (i + 1) * B_PER_CORE].reshape(ROWS, W, C)}
        for i in range(N_CORES)
    ]
    res = run_bass_kernel_spmd(nc, in_maps, core_ids=list(range(N_CORES)), **kwargs)
    outs = [r["y"].reshape(B_PER_CORE, H, W, C) for r in res.results]
    out = np.concatenate(outs, axis=0)
    return out, res


def kernel(x: np.ndarray) -> np.ndarray:
    out, _ = run(x)
    return out


def _build_fn1(nc):
    """Single-execution jit over an 8-core mesh (one bass_exec per module —
    the bass2jax hook rejects modules with more than one)."""
    import jax
    from jax.sharding import Mesh, PartitionSpec
    from jax.experimental.shard_map import shard_map

    from concourse import bass2jax
    from concourse import mybir as _mybir

    bass2jax.install_neuronx_cc_hook()

    partition_name = (
        nc.partition_id_tensor.name if nc.partition_id_tensor is not None else None
    )
    in_names, out_names, out_avals = [], [], []
    for alloc in nc.m.functions[0].allocations:
        if not isinstance(alloc, _mybir.MemoryLocationSet):
            continue
        name = alloc.memorylocations[0].name
        if alloc.kind == "ExternalInput":
            if name != partition_name:
                in_names.append(name)
        elif alloc.kind == "ExternalOutput":
            out_names.append(name)
            out_avals.append(
                jax.core.ShapedArray(
                    tuple(alloc.tensor_shape), _mybir.dt.np(alloc.dtype)
                )
            )
    n_params = len(in_names)
    all_names = in_names + out_names
    if partition_name is not None:
        all_names = all_names + [partition_name]

    def _body(*args):
        operands = list(args)
        if partition_name is not None:
            operands.append(bass2jax.partition_id_tensor())
        outs = bass2jax._bass_exec_p.bind(
            *operands,
            out_avals=tuple(out_avals),
            in_names=tuple(all_names),
            out_names=tuple(out_names),
            lowering_input_output_aliases=(),
            sim_require_finite=True,
            sim_require_nnan=True,
            nc=nc,
        )
        return tuple(outs)

    devices = jax.devices()[:N_CORES]
    mesh = Mesh(np.asarray(devices), ("core",))
    nspec = n_params + len(out_names)
    fn1 = jax.jit(
        shard_map(
            _body,
            mesh=mesh,
            in_specs=(PartitionSpec("core"),) * nspec,
            out_specs=(PartitionSpec("core"),) * len(out_names),
            check_rep=False,
        ),
        keep_unused=True,
    )
    sharding = jax.sharding.NamedSharding(mesh, PartitionSpec("core"))
    return fn1, sharding


def bench(x: np.ndarray, n_lo: int = 8, n_hi: int = 136, reps: int = 4) -> dict:
    """Measure steady-state device time per kernel execution.

    Executions are chained ACROSS jit calls (call k+1 consumes call k's
    output as its output-init operand): device memory stays constant, the
    executions serialize on-device, and total wall time is
    fixed_tunnel_latency + n * per_exec_time. The (n_lo, n_hi) slope then
    estimates per-execution time with the tunnel latency cancelled. This is
    an upper bound on pure device time (per-execution dispatch overhead that
    does not pipeline is included).
    """
    import jax

    x = np.ascontiguousarray(x, dtype=np.float32)
    nc = _get_nc()
    fn1, sharding = _build_fn1(nc)

    xg = x.reshape(N_CORES * ROWS, W, C)
    x_dev = jax.device_put(xg, sharding)
    out_dev = jax.device_put(np.zeros_like(xg), sharding)

    def chain(n):
        t0 = time.perf_counter()
        out = out_dev
        for _ in range(n):
            out = fn1(x_dev, out)[0]
        jax.block_until_ready(out)
        return time.perf_counter() - t0, out

    _, out = chain(2)  # warmup (compile + caches)
    result = np.asarray(out).reshape(B, H, W, C)

    best = float("inf")
    t_lo_b = t_hi_b = None
    for _ in range(reps):
        t_lo, _ = chain(n_lo)
        t_hi, _ = chain(n_hi)
        slope = (t_hi - t_lo) / (n_hi - n_lo) * 1e9
        if slope < best:
            best, t_lo_b, t_hi_b = slope, t_lo, t_hi

    return {
        "device_ns": best,
        "t_lo_ns": t_lo_b * 1e9,
        "t_hi_ns": t_hi_b * 1e9,
        "n_lo": n_lo,
        "n_hi": n_hi,
        "out": result,
    }
